# revision 6
# baseline (speedup 1.0000x reference)
"""Trainium2 Bass kernel for AttentionalPlanarRemapping.

out[n,c,h,w] = sum_d softmax(atts[n,c,:])[d] * images[n,d,h,w]

Per-sample: W = softmax(atts[n]) [C,C]; out[n] = W @ images[n].reshape(C, H*W).

Sharding: data-parallel over N across 8 cores (4 samples per core).

Host preprocessing inside kernel(): both inputs are cast to fp16 and
pre-shuffled into the exact SBUF tile layout [P=128 partitions, KD, free]:
  x[n, p, kd, hw] = images[n, kd*128+p, hw]     (8KB contiguous/partition)
  a[n, p, kd, c]  = atts[n, c, kd*128+p]        (transposed: lhsT layout)
so every DMA descriptor is a 2-8KB contiguous run (the v2 trace showed atts
loads crawling at 38-110 GB/s on 1KB descriptors) and no on-device
transposition is needed. fp16 matches the PE's full-rate matmul path while
halving DMA traffic; |atts| < 6 so exp needs no max-subtraction. (fp8 was
measured offline: e4m3 on both operands gives rel_err 5.6e-2 > the 2e-2
budget, so DoubleRow is not available accuracy-wise.)

Per-core plan (v3 -- from NTFF trace analysis of v1@58.5us, v2@51.2us):
  - The early phase is HBM-supply-bound (packets start ~2.3us after the
    start barrier, ramp to ~390 GB/s) and the PE HAM clock gate needs
    ~3.4us of sustained busy to reach 2.4 GHz. 30 dummy N=128 matmuls on a
    memset tile keep the PE busy from ~7.2us with no idle window, so HAM
    hits 8/8 right before the first real matmul's data lands (~11.5us) and
    every real matmul runs at full clock.
  - Tile software-pipelines each block's LDWEIGHTS ahead of the previous
    block's matmuls, and a semaphore-blocked LDWEIGHTS head-of-line blocks
    the strict-FIFO PE queue (v2 lost 4.4us to LDW(kd1) waiting on the kd1
    exp chunk). So the sample-0 exp chain must complete BEFORE the matmul
    stream reaches it: a0 rides both HWDGE queues as kd-pair halves at
    queue head, and the exp chunks are gated on those fine receipts.
  - Loads ride the two HWDGE queues (sync=SP, scalar=ACT) in consumption
    order; later-needed tensors queue BEHIND earlier ones so they cannot
    starve the critical path. gpsimd (SWDGE, ~131 GB/s ceiling) carries
    only deferred stores:
        scalar: a0(kd01), x0(kd1), x0(kd3)         then the exp stream
        sync:   a0(kd23), x0(kd0), x0(kd2), x1, a1, x2, a2, a3, x3
    x1..x3/a1..a3 are single full-tile triggers (4-8KB descriptors).
  - Main matmuls run on UNNORMALIZED E = exp(a): for each output block kc
    (128 rows of c) and each psum half ht, accumulate over kd:
        ps[ht][c128, 512] += E[kd-blk, kc-blk].T @ X[kd-blk, ht-half]
    Each [128,512] f32 psum tile is exactly one PSUM bank; 6 bufs rotate.
  - Softmax denominators ride along as tiny N=2 matmuls on the SAME loaded
    weights (sm[kc] += E.T @ ones), emitted before the two 512-col matmuls
    of each kd so the kd=3 stop retires early; r = 1/s via DVE reciprocal;
    evictions scale by r while casting to fp16 (ht0 on ACT for kc<2 else
    DVE; ht1 on DVE) -- at most one pending exp chunk can ever
    head-of-line block an ACT eviction.
  - exp(n+1) is emitted one kd-chunk per kc block of compute(n).
  - Stores are deferred by one sample and alternate queues (n0,n2 ->
    gpsimd/SWDGE; n1 -> sync, idle once the loads drain). Sample 3 stores
    kc0-2 on sync immediately after each eviction; the final kc3 block
    evicts in PSUM-bank quarters fanned across ACT/DVE, each quarter
    stored immediately on its own queue (sync/gpsimd/scalar/sync) to
    shorten the last-matmul -> last-byte chain.

Measured: v1 50-58.5us (chip-power dependent), v2 51.2us. v3 target ~46us.
"""

import numpy as np
from contextlib import ExitStack

import concourse.bass as bass
import concourse.mybir as mybir
import concourse.tile as tile
from concourse import bacc
from concourse.bass_utils import run_bass_kernel_spmd

N, C, H, W = 32, 512, 32, 32
HW = H * W                      # 1024
NCORES = 8
NPC = N // NCORES               # 4 samples per core
P = 128
KC = C // P                     # 4 chunks over output channel c
KD = C // P                     # 4 chunks over contraction d
NT = 512                        # matmul moving free dim (one PSUM bank of f32)
NHT = HW // NT                  # 2
NWARM = 38                      # dummy N=128 matmuls: >3.4us of PE busy

F32 = mybir.dt.float32
F16 = mybir.dt.float16
AF = mybir.ActivationFunctionType


def build_nc():
    nc = bacc.Bacc("TRN2", target_bir_lowering=False, debug=False)

    # both inputs pre-shuffled on host into SBUF tile layout [P, KD, free]
    images = nc.dram_tensor("images", [NPC, P, KD, HW], F16, kind="ExternalInput").ap()
    attsT = nc.dram_tensor("attsT", [NPC, P, KD, C], F16, kind="ExternalInput").ap()
    out = nc.dram_tensor("out", [NPC, C, HW], F16, kind="ExternalOutput").ap()

    with ExitStack() as ctx:
        tc = ctx.enter_context(tile.TileContext(nc))

        const_pool = ctx.enter_context(tc.tile_pool(name="const", bufs=1))
        ones_f32 = const_pool.tile([P, 2], F32)
        ones = const_pool.tile([P, 2], F16)
        warm_x = const_pool.tile([P, P], F16)

        a_pool = ctx.enter_context(tc.tile_pool(name="a", bufs=1))
        e_pool = ctx.enter_context(tc.tile_pool(name="e", bufs=1))
        x_pool = ctx.enter_context(tc.tile_pool(name="x", bufs=1))
        o_pool = ctx.enter_context(tc.tile_pool(name="o", bufs=8))
        r_pool = ctx.enter_context(tc.tile_pool(name="r", bufs=2))
        mm_psum = ctx.enter_context(tc.tile_pool(name="mmp", bufs=6, space="PSUM"))
        sm_psum = ctx.enter_context(tc.tile_pool(name="smp", bufs=2, space="PSUM"))

        # ---- constants (gpsimd; cheap, before its store-only stream) ----
        nc.gpsimd.memset(warm_x[:], 0.0)
        nc.gpsimd.memset(ones_f32[:], 1.0)
        nc.vector.tensor_copy(ones[:], ones_f32[:])

        a_tiles = []
        x_tiles = []
        for n in range(NPC):
            a_tiles.append(a_pool.tile([P, KD, C], F16, name=f"a{n}", tag=f"a{n}"))
            x_tiles.append(x_pool.tile([P, KD, HW], F16, name=f"x{n}", tag=f"x{n}"))

        # ---- input loads: 2 HWDGE queues, landing order == consumption
        # order; sample-0 tensors split in fine chunks across BOTH queue
        # heads; gpsimd (SWDGE) carries no loads ----
        # The SDMA engines round-robin packets across ALL in-flight DMAs,
        # so bulk loads issued early steal bandwidth from the sample-0
        # critical path. Tile rotates a pool of 8 DMA-completion
        # semaphores and makes each reuse WAIT for the previous user, so
        # after 6 sample-0 triggers we burn the 2 remaining fresh sems on
        # tiny decoy transfers -- every bulk trigger below then inherits a
        # reused semaphore and only enters flight once a sample-0 chunk
        # has fully landed.
        nc.scalar.dma_start(a_tiles[0][:, 0:2], attsT[0][:, 0:2])
        nc.sync.dma_start(a_tiles[0][:, 2:4], attsT[0][:, 2:4])
        nc.sync.dma_start(x_tiles[0][:, 0:1], images[0][:, 0:1])
        nc.scalar.dma_start(x_tiles[0][:, 1:2], images[0][:, 1:2])
        nc.sync.dma_start(x_tiles[0][:, 2:3], images[0][:, 2:3])
        nc.scalar.dma_start(x_tiles[0][:, 3:4], images[0][:, 3:4])
        decoy = const_pool.tile([1, 2, 64], F16)
        nc.sync.dma_start(decoy[:, 0:1], attsT[0][0:1, 0:1, 0:64])
        nc.sync.dma_start(decoy[:, 1:2], attsT[0][0:1, 1:2, 0:64])
        nc.sync.dma_start(x_tiles[1][:], images[1][:])
        nc.sync.dma_start(a_tiles[1][:], attsT[1][:])
        nc.sync.dma_start(x_tiles[2][:], images[2][:])
        nc.sync.dma_start(a_tiles[2][:], attsT[2][:])
        nc.sync.dma_start(a_tiles[3][:], attsT[3][:])
        nc.sync.dma_start(x_tiles[3][:], images[3][:])

        # ---- exp ----
        e_tiles = [e_pool.tile([P, KD, C], F16, name=f"e{n}", tag=f"e{n}") for n in range(NPC)]

        def emit_exp_chunk(n, kd):
            """exp of one kd chunk of sample n (ACT, ~0.7us each)."""
            nc.scalar.activation(
                e_tiles[n][:, kd : kd + 1],
                a_tiles[n][:, kd : kd + 1],
                AF.Exp,
                bias=0.0,
                scale=1.0,
            )

        # first slab covers only (kd=0, kc=0)'s weights so the very first
        # matmul is gated by the DMA receipt, not a full exp chunk
        nc.scalar.activation(
            e_tiles[0][:, 0, 0:P], a_tiles[0][:, 0, 0:P], AF.Exp, bias=0.0, scale=1.0
        )
        nc.scalar.activation(
            e_tiles[0][:, 0, P:], a_tiles[0][:, 0, P:], AF.Exp, bias=0.0, scale=1.0
        )
        for kd in range(1, KD):
            emit_exp_chunk(0, kd)

        # ---- PE warm-up: NWARM dummy N=128 matmuls keep the PE busy from
        # ~7.2us with no idle window, so the HAM clock gate reaches 8/8 at
        # ~10.6us -- right before the first real matmul's data lands. Fine
        # N=128 granularity bounds the cost of leftover dummies.
        warm_ps = mm_psum.tile([P, NT], F32, name="warm_ps", tag="ps", space="PSUM")
        for _ in range(NWARM):
            nc.tensor.matmul(
                warm_ps[0:2, 0:P],
                lhsT=warm_x[:, 0:2],
                rhs=warm_x[:],
                start=True,
                stop=True,
            )

        # Stores are deferred by one sample: sample n's stores are emitted
        # during compute(n+1), after the loads have drained, alternating
        # between the SWDGE (gpsimd) queue and the by-then-idle sync queue.
        pending_stores = []

        def compute(n):
            e_t = e_tiles[n]
            x_t = x_tiles[n]
            r_sb = r_pool.tile([P, KC], F32, name=f"r{n}", tag="r")
            for kc in range(KC):
                if pending_stores:
                    dram_ap, o_prev, q = pending_stores.pop(0)
                    if q == "gpsimd":
                        nc.gpsimd.dma_start(dram_ap, o_prev)
                    else:
                        nc.sync.dma_start(dram_ap, o_prev)
                ps = [
                    mm_psum.tile(
                        [P, NT], F32, name=f"ps{n}_{kc}_{ht}", tag="ps", space="PSUM"
                    )
                    for ht in range(NHT)
                ]
                sm = sm_psum.tile([P, 2], F32, name=f"sm{n}_{kc}", tag="sm",
                                  space="PSUM")
                for kd in range(KD):
                    lhs = e_t[:, kd, kc * P : (kc + 1) * P]
                    # tiny sum-matmul first: its kd=3 stop gates the
                    # reciprocal, so retiring it before the two 512-col
                    # matmuls shortens the eviction critical path
                    nc.tensor.matmul(
                        sm[:],
                        lhsT=lhs,
                        rhs=ones[:],
                        start=(kd == 0),
                        stop=(kd == KD - 1),
                    )
                    for ht in range(NHT):
                        nc.tensor.matmul(
                            ps[ht][:],
                            lhsT=lhs,
                            rhs=x_t[:, kd, ht * NT : (ht + 1) * NT],
                            start=(kd == 0),
                            stop=(kd == KD - 1),
                        )
                r_ap = r_sb[:, kc : kc + 1]
                nc.vector.reciprocal(r_ap, sm[:, 0:1])
                o_t = o_pool.tile([P, HW], F16, name=f"o{n}_{kc}", tag="o")
                last = n == NPC - 1 and kc == KC - 1
                if last:
                    # quarter-granular evict fan-out across ACT/DVE, each
                    # quarter stored immediately on its own (idle) queue:
                    # shortens the last-matmul -> last-byte critical chain
                    NQ = NT // 2
                    orow = out[n][kc * P : (kc + 1) * P]
                    nc.scalar.mul(o_t[:, 0:NQ], ps[0][:, 0:NQ], r_ap)
                    nc.vector.tensor_scalar_mul(o_t[:, NQ:NT], ps[0][:, NQ:], r_ap)
                    nc.sync.dma_start(orow[:, 0:NQ], o_t[:, 0:NQ])
                    nc.gpsimd.dma_start(orow[:, NQ:NT], o_t[:, NQ:NT])
                    nc.scalar.mul(o_t[:, NT : NT + NQ], ps[1][:, 0:NQ], r_ap)
                    nc.vector.tensor_scalar_mul(o_t[:, NT + NQ :], ps[1][:, NQ:], r_ap)
                    nc.scalar.dma_start(orow[:, NT : NT + NQ], o_t[:, NT : NT + NQ])
                    nc.sync.dma_start(orow[:, NT + NQ :], o_t[:, NT + NQ :])
                else:
                    # ht0 evictions for kc>=2 go to DVE: an exp chunk stuck
                    # on a late DMA receipt head-of-line blocks the ACT
                    # queue, and evictions parked behind it hold PSUM banks
                    # the PE is waiting for. DVE has slack and carries no
                    # exp, so only kc0/kc1's evictions can ever be blocked.
                    if kc < 2:
                        nc.scalar.mul(o_t[:, 0:NT], ps[0][:], r_ap)
                    else:
                        nc.vector.tensor_scalar_mul(o_t[:, 0:NT], ps[0][:], r_ap)
                    nc.vector.tensor_scalar_mul(o_t[:, NT:], ps[1][:], r_ap)
                    if n == NPC - 1:
                        # last sample's early stores go on the (idle) sync
                        # queue immediately after each eviction
                        nc.sync.dma_start(out[n][kc * P : (kc + 1) * P], o_t[:])
                    else:
                        q = "sync" if n == 1 else "gpsimd"
                        pending_stores.append(
                            (out[n][kc * P : (kc + 1) * P], o_t, q)
                        )
                if n + 1 < NPC:
                    emit_exp_chunk(n + 1, kc)

        for n in range(NPC):
            compute(n)

    nc.compile()
    return nc


_NC_CACHE = None


def _get_nc():
    global _NC_CACHE
    if _NC_CACHE is None:
        _NC_CACHE = build_nc()
    return _NC_CACHE


def run(in_maps, **kwargs):
    """Run the SPMD kernel on cores 0..7. in_maps: one dict per core."""
    nc = _get_nc()
    return run_bass_kernel_spmd(nc, in_maps, core_ids=list(range(NCORES)), **kwargs)


def make_in_maps(images: np.ndarray, atts: np.ndarray):
    images = np.asarray(images, dtype=np.float32).astype(np.float16)
    atts = np.asarray(atts, dtype=np.float32)
    assert images.shape == (N, C, H, W), images.shape
    assert atts.shape == (N, C, C), atts.shape
    # x[i, n, p, kd, hw] = images[i, n, kd*128+p, hw]
    img_s = (
        images.reshape(NCORES, NPC, KD, P, HW)
        .transpose(0, 1, 3, 2, 4)
    )
    # a[i, n, p, kd, c] = atts[i*NPC+n, c, kd*128+p]  (per-sample transpose)
    attsT = (
        atts.transpose(0, 2, 1)
        .astype(np.float16)
        .reshape(NCORES, NPC, KD, P, C)
        .transpose(0, 1, 3, 2, 4)
    )
    return [
        {
            "images": np.ascontiguousarray(img_s[i]),
            "attsT": np.ascontiguousarray(attsT[i]),
        }
        for i in range(NCORES)
    ]


def kernel(images: np.ndarray, atts: np.ndarray) -> np.ndarray:
    in_maps = make_in_maps(images, atts)
    res = run(in_maps)
    outs = [res.results[i]["out"] for i in range(NCORES)]
    full = np.concatenate(outs, axis=0).reshape(N, C, H, W)
    return full.astype(np.float32)


# revision 9
# speedup vs baseline: 1.0220x; 1.0220x over previous
"""Trainium2 Bass kernel for AttentionalPlanarRemapping.

out[n,c,h,w] = sum_d softmax(atts[n,c,:])[d] * images[n,d,h,w]

Per-sample: W = softmax(atts[n]) [C,C]; out[n] = W @ images[n].reshape(C, H*W).

Sharding: data-parallel over N across 8 cores (4 samples per core).

Host preprocessing inside kernel(): both inputs are cast to fp16 and
pre-shuffled into the exact SBUF tile layout [P=128 partitions, KD, free]:
  x[n, p, kd, hw] = images[n, kd*128+p, hw]     (8KB contiguous/partition)
  a[n, p, kd, c]  = atts[n, c, kd*128+p]        (transposed: lhsT layout)
so every DMA descriptor is a 2-8KB contiguous run (the v2 trace showed atts
loads crawling at 38-110 GB/s on 1KB descriptors) and no on-device
transposition is needed. fp16 matches the PE's full-rate matmul path while
halving DMA traffic; |atts| < 6 so exp needs no max-subtraction. (fp8 was
measured offline: e4m3 on both operands gives rel_err 5.6e-2 > the 2e-2
budget, so DoubleRow is not available accuracy-wise.)

Per-core plan (v3 -- from NTFF trace analysis of v1@58.5us, v2@51.2us):
  - The early phase is HBM-supply-bound (packets start ~2.3us after the
    start barrier, ramp to ~390 GB/s) and the PE HAM clock gate needs
    ~3.4us of sustained busy to reach 2.4 GHz. 30 dummy N=128 matmuls on a
    memset tile keep the PE busy from ~7.2us with no idle window, so HAM
    hits 8/8 right before the first real matmul's data lands (~11.5us) and
    every real matmul runs at full clock.
  - Tile software-pipelines each block's LDWEIGHTS ahead of the previous
    block's matmuls, and a semaphore-blocked LDWEIGHTS head-of-line blocks
    the strict-FIFO PE queue (v2 lost 4.4us to LDW(kd1) waiting on the kd1
    exp chunk). So the sample-0 exp chain must complete BEFORE the matmul
    stream reaches it: a0 rides both HWDGE queues as kd-pair halves at
    queue head, and the exp chunks are gated on those fine receipts.
  - Loads ride the two HWDGE queues (sync=SP, scalar=ACT) in consumption
    order; later-needed tensors queue BEHIND earlier ones so they cannot
    starve the critical path. gpsimd (SWDGE, ~131 GB/s ceiling) carries
    only deferred stores:
        scalar: a0(kd01), x0(kd1), x0(kd3)         then the exp stream
        sync:   a0(kd23), x0(kd0), x0(kd2), x1, a1, x2, a2, a3, x3
    x1..x3/a1..a3 are single full-tile triggers (4-8KB descriptors).
  - Main matmuls run on UNNORMALIZED E = exp(a): for each output block kc
    (128 rows of c) and each psum half ht, accumulate over kd:
        ps[ht][c128, 512] += E[kd-blk, kc-blk].T @ X[kd-blk, ht-half]
    Each [128,512] f32 psum tile is exactly one PSUM bank; 6 bufs rotate.
  - Softmax denominators ride along as tiny N=2 matmuls on the SAME loaded
    weights (sm[kc] += E.T @ ones), emitted before the two 512-col matmuls
    of each kd so the kd=3 stop retires early; r = 1/s via DVE reciprocal;
    evictions scale by r while casting to fp16 (ht0 on ACT for kc<2 else
    DVE; ht1 on DVE) -- at most one pending exp chunk can ever
    head-of-line block an ACT eviction.
  - exp(n+1) is emitted one kd-chunk per kc block of compute(n).
  - Stores are deferred by one sample and alternate queues (n0,n2 ->
    gpsimd/SWDGE; n1 -> sync, idle once the loads drain). Sample 3 stores
    kc0-2 on sync immediately after each eviction; the final kc3 block
    evicts in PSUM-bank quarters fanned across ACT/DVE, each quarter
    stored immediately on its own queue (sync/gpsimd/scalar/sync) to
    shorten the last-matmul -> last-byte chain.

Measured: v1 50-58.5us (chip-power dependent), v2 51.2us. v3 target ~46us.
"""

import numpy as np
from contextlib import ExitStack

import concourse.bass as bass
import concourse.mybir as mybir
import concourse.tile as tile
from concourse import bacc
from concourse.bass_utils import run_bass_kernel_spmd

N, C, H, W = 32, 512, 32, 32
HW = H * W                      # 1024
NCORES = 8
NPC = N // NCORES               # 4 samples per core
P = 128
KC = C // P                     # 4 chunks over output channel c
KD = C // P                     # 4 chunks over contraction d
NT = 512                        # matmul moving free dim (one PSUM bank of f32)
NHT = HW // NT                  # 2
NWARM = 44                      # dummy N=128 matmuls: >3.4us of PE busy

F32 = mybir.dt.float32
F16 = mybir.dt.float16
AF = mybir.ActivationFunctionType


def build_nc():
    nc = bacc.Bacc("TRN2", target_bir_lowering=False, debug=False)

    # both inputs pre-shuffled on host into SBUF tile layout [P, KD, free]
    images = nc.dram_tensor("images", [NPC, P, KD, HW], F16, kind="ExternalInput").ap()
    attsT = nc.dram_tensor("attsT", [NPC, P, KD, C], F16, kind="ExternalInput").ap()
    out = nc.dram_tensor("out", [NPC, C, HW], F16, kind="ExternalOutput").ap()

    with ExitStack() as ctx:
        tc = ctx.enter_context(tile.TileContext(nc))

        const_pool = ctx.enter_context(tc.tile_pool(name="const", bufs=1))
        ones_f32 = const_pool.tile([P, 2], F32)
        ones = const_pool.tile([P, 2], F16)
        warm_x = const_pool.tile([P, P], F16)

        a_pool = ctx.enter_context(tc.tile_pool(name="a", bufs=1))
        e_pool = ctx.enter_context(tc.tile_pool(name="e", bufs=1))
        x_pool = ctx.enter_context(tc.tile_pool(name="x", bufs=1))
        o_pool = ctx.enter_context(tc.tile_pool(name="o", bufs=8))
        r_pool = ctx.enter_context(tc.tile_pool(name="r", bufs=2))
        mm_psum = ctx.enter_context(tc.tile_pool(name="mmp", bufs=6, space="PSUM"))
        sm_psum = ctx.enter_context(tc.tile_pool(name="smp", bufs=2, space="PSUM"))

        # ---- constants (gpsimd; cheap, before its store-only stream) ----
        nc.gpsimd.memset(warm_x[:], 0.0)
        nc.gpsimd.memset(ones_f32[:], 1.0)
        nc.vector.tensor_copy(ones[:], ones_f32[:])

        a_tiles = []
        x_tiles = []
        for n in range(NPC):
            a_tiles.append(a_pool.tile([P, KD, C], F16, name=f"a{n}", tag=f"a{n}"))
            x_tiles.append(x_pool.tile([P, KD, HW], F16, name=f"x{n}", tag=f"x{n}"))

        # ---- input loads: 2 HWDGE queues, landing order == consumption
        # order; sample-0 tensors split in fine chunks across BOTH queue
        # heads; gpsimd (SWDGE) carries no loads ----
        # The SDMA engines round-robin packets across ALL in-flight DMAs,
        # so bulk loads issued early steal bandwidth from the sample-0
        # critical path. Tile rotates a pool of 8 DMA-completion
        # semaphores and makes each reuse WAIT for the previous user, so
        # after 6 sample-0 triggers we burn the 2 remaining fresh sems on
        # tiny decoy transfers -- every bulk trigger below then inherits a
        # reused semaphore and only enters flight once a sample-0 chunk
        # has fully landed.
        nc.scalar.dma_start(a_tiles[0][:, 0:2], attsT[0][:, 0:2])
        nc.sync.dma_start(x_tiles[0][:, 0:1], images[0][:, 0:1])
        nc.sync.dma_start(a_tiles[0][:, 2:4], attsT[0][:, 2:4])
        nc.scalar.dma_start(x_tiles[0][:, 1:2], images[0][:, 1:2])
        nc.sync.dma_start(x_tiles[0][:, 2:3], images[0][:, 2:3])
        nc.scalar.dma_start(x_tiles[0][:, 3:4], images[0][:, 3:4])
        decoy = const_pool.tile([1, 2, 64], F16)
        nc.sync.dma_start(decoy[:, 0:1], attsT[0][0:1, 0:1, 0:64])
        nc.sync.dma_start(decoy[:, 1:2], attsT[0][0:1, 1:2, 0:64])
        nc.sync.dma_start(x_tiles[1][:], images[1][:])
        nc.sync.dma_start(a_tiles[1][:], attsT[1][:])
        nc.sync.dma_start(x_tiles[2][:], images[2][:])
        nc.sync.dma_start(a_tiles[2][:], attsT[2][:])
        nc.sync.dma_start(a_tiles[3][:], attsT[3][:])
        nc.sync.dma_start(x_tiles[3][:], images[3][:])

        # ---- exp ----
        e_tiles = [e_pool.tile([P, KD, C], F16, name=f"e{n}", tag=f"e{n}") for n in range(NPC)]

        def emit_exp_chunk(n, kd):
            """exp of one kd chunk of sample n (ACT, ~0.7us each)."""
            nc.scalar.activation(
                e_tiles[n][:, kd : kd + 1],
                a_tiles[n][:, kd : kd + 1],
                AF.Exp,
                bias=0.0,
                scale=1.0,
            )

        # first slab covers only (kd=0, kc=0)'s weights so the very first
        # matmul is gated by the DMA receipt, not a full exp chunk
        nc.scalar.activation(
            e_tiles[0][:, 0, 0:P], a_tiles[0][:, 0, 0:P], AF.Exp, bias=0.0, scale=1.0
        )
        nc.scalar.activation(
            e_tiles[0][:, 0, P:], a_tiles[0][:, 0, P:], AF.Exp, bias=0.0, scale=1.0
        )
        for kd in range(1, KD):
            emit_exp_chunk(0, kd)

        # ---- PE warm-up: NWARM dummy N=128 matmuls keep the PE busy from
        # ~7.2us with no idle window, so the HAM clock gate reaches 8/8 at
        # ~10.6us -- right before the first real matmul's data lands. Fine
        # N=128 granularity bounds the cost of leftover dummies.
        warm_ps = mm_psum.tile([P, NT], F32, name="warm_ps", tag="ps", space="PSUM")
        for _ in range(NWARM):
            nc.tensor.matmul(
                warm_ps[0:2, 0:P],
                lhsT=warm_x[:, 0:2],
                rhs=warm_x[:],
                start=True,
                stop=True,
            )

        # Stores are deferred by one sample: sample n's stores are emitted
        # during compute(n+1), after the loads have drained, alternating
        # between the SWDGE (gpsimd) queue and the by-then-idle sync queue.
        pending_stores = []

        def compute(n):
            e_t = e_tiles[n]
            x_t = x_tiles[n]
            r_sb = r_pool.tile([P, KC], F32, name=f"r{n}", tag="r")
            for kc in range(KC):
                if pending_stores:
                    dram_ap, o_prev, q = pending_stores.pop(0)
                    if q == "gpsimd":
                        nc.gpsimd.dma_start(dram_ap, o_prev)
                    else:
                        nc.sync.dma_start(dram_ap, o_prev)
                last = n == NPC - 1 and kc == KC - 1
                sm = sm_psum.tile([P, 2], F32, name=f"sm{n}_{kc}", tag="sm",
                                  space="PSUM")
                NQ = NT // 2
                if last:
                    # the final block accumulates ht1 as two N=256 groups
                    # so the last group's stop (and thus the last eviction
                    # + store) retires with a quarter-tile, shortening the
                    # last-matmul -> last-byte critical chain
                    ps0 = mm_psum.tile([P, NT], F32, name=f"ps{n}_{kc}_0",
                                       tag="ps", space="PSUM")
                    ps1a = mm_psum.tile([P, NQ], F32, name=f"ps{n}_{kc}_1a",
                                        tag="ps", space="PSUM")
                    ps1b = mm_psum.tile([P, NQ], F32, name=f"ps{n}_{kc}_1b",
                                        tag="ps", space="PSUM")
                    for kd in range(KD):
                        lhs = e_t[:, kd, kc * P : (kc + 1) * P]
                        st, sp = kd == 0, kd == KD - 1
                        nc.tensor.matmul(sm[:], lhsT=lhs, rhs=ones[:],
                                         start=st, stop=sp)
                        nc.tensor.matmul(ps0[:], lhsT=lhs,
                                         rhs=x_t[:, kd, 0:NT], start=st, stop=sp)
                        nc.tensor.matmul(ps1a[:], lhsT=lhs,
                                         rhs=x_t[:, kd, NT : NT + NQ],
                                         start=st, stop=sp)
                        nc.tensor.matmul(ps1b[:], lhsT=lhs,
                                         rhs=x_t[:, kd, NT + NQ :],
                                         start=st, stop=sp)
                    r_ap = r_sb[:, kc : kc + 1]
                    nc.vector.reciprocal(r_ap, sm[:, 0:1])
                    o_t = o_pool.tile([P, HW], F16, name=f"o{n}_{kc}", tag="o")
                    orow = out[n][kc * P : (kc + 1) * P]
                    # quarter-granular evict fan-out across ACT/DVE, each
                    # quarter stored immediately on its own (idle) queue
                    nc.scalar.mul(o_t[:, 0:NQ], ps0[:, 0:NQ], r_ap)
                    nc.vector.tensor_scalar_mul(o_t[:, NQ:NT], ps0[:, NQ:], r_ap)
                    nc.sync.dma_start(orow[:, 0:NQ], o_t[:, 0:NQ])
                    nc.gpsimd.dma_start(orow[:, NQ:NT], o_t[:, NQ:NT])
                    nc.scalar.mul(o_t[:, NT : NT + NQ], ps1a[:], r_ap)
                    nc.vector.tensor_scalar_mul(o_t[:, NT + NQ :], ps1b[:], r_ap)
                    nc.scalar.dma_start(orow[:, NT : NT + NQ], o_t[:, NT : NT + NQ])
                    nc.sync.dma_start(orow[:, NT + NQ :], o_t[:, NT + NQ :])
                    if n + 1 < NPC:
                        emit_exp_chunk(n + 1, kc)
                    continue
                ps = [
                    mm_psum.tile(
                        [P, NT], F32, name=f"ps{n}_{kc}_{ht}", tag="ps", space="PSUM"
                    )
                    for ht in range(NHT)
                ]
                for kd in range(KD):
                    lhs = e_t[:, kd, kc * P : (kc + 1) * P]
                    # tiny sum-matmul first: its kd=3 stop gates the
                    # reciprocal, so retiring it before the two 512-col
                    # matmuls shortens the eviction critical path
                    nc.tensor.matmul(
                        sm[:],
                        lhsT=lhs,
                        rhs=ones[:],
                        start=(kd == 0),
                        stop=(kd == KD - 1),
                    )
                    for ht in range(NHT):
                        nc.tensor.matmul(
                            ps[ht][:],
                            lhsT=lhs,
                            rhs=x_t[:, kd, ht * NT : (ht + 1) * NT],
                            start=(kd == 0),
                            stop=(kd == KD - 1),
                        )
                r_ap = r_sb[:, kc : kc + 1]
                nc.vector.reciprocal(r_ap, sm[:, 0:1])
                o_t = o_pool.tile([P, HW], F16, name=f"o{n}_{kc}", tag="o")
                if True:
                    # ht0 evictions for kc>=2 go to DVE: an exp chunk stuck
                    # on a late DMA receipt head-of-line blocks the ACT
                    # queue, and evictions parked behind it hold PSUM banks
                    # the PE is waiting for. DVE has slack and carries no
                    # exp, so only kc0/kc1's evictions can ever be blocked.
                    if kc < 2:
                        nc.scalar.mul(o_t[:, 0:NT], ps[0][:], r_ap)
                    else:
                        nc.vector.tensor_scalar_mul(o_t[:, 0:NT], ps[0][:], r_ap)
                    nc.vector.tensor_scalar_mul(o_t[:, NT:], ps[1][:], r_ap)
                    if n == NPC - 1:
                        # last sample's early stores go on the (idle) sync
                        # queue immediately after each eviction
                        nc.sync.dma_start(out[n][kc * P : (kc + 1) * P], o_t[:])
                    else:
                        q = "sync" if n == 1 else "gpsimd"
                        pending_stores.append(
                            (out[n][kc * P : (kc + 1) * P], o_t, q)
                        )
                if n + 1 < NPC:
                    emit_exp_chunk(n + 1, kc)

        for n in range(NPC):
            compute(n)

    nc.compile()
    return nc


_NC_CACHE = None


def _get_nc():
    global _NC_CACHE
    if _NC_CACHE is None:
        _NC_CACHE = build_nc()
    return _NC_CACHE


def run(in_maps, **kwargs):
    """Run the SPMD kernel on cores 0..7. in_maps: one dict per core."""
    nc = _get_nc()
    return run_bass_kernel_spmd(nc, in_maps, core_ids=list(range(NCORES)), **kwargs)


def make_in_maps(images: np.ndarray, atts: np.ndarray):
    images = np.asarray(images, dtype=np.float32).astype(np.float16)
    atts = np.asarray(atts, dtype=np.float32)
    assert images.shape == (N, C, H, W), images.shape
    assert atts.shape == (N, C, C), atts.shape
    # x[i, n, p, kd, hw] = images[i, n, kd*128+p, hw]
    img_s = (
        images.reshape(NCORES, NPC, KD, P, HW)
        .transpose(0, 1, 3, 2, 4)
    )
    # a[i, n, p, kd, c] = atts[i*NPC+n, c, kd*128+p]  (per-sample transpose)
    attsT = (
        atts.transpose(0, 2, 1)
        .astype(np.float16)
        .reshape(NCORES, NPC, KD, P, C)
        .transpose(0, 1, 3, 2, 4)
    )
    return [
        {
            "images": np.ascontiguousarray(img_s[i]),
            "attsT": np.ascontiguousarray(attsT[i]),
        }
        for i in range(NCORES)
    ]


def kernel(images: np.ndarray, atts: np.ndarray) -> np.ndarray:
    in_maps = make_in_maps(images, atts)
    res = run(in_maps)
    outs = [res.results[i]["out"] for i in range(NCORES)]
    full = np.concatenate(outs, axis=0).reshape(N, C, H, W)
    return full.astype(np.float32)
